# revision 35
# baseline (speedup 1.0000x reference)
"""Trainium2 Bass kernel for nn_AtomicConvScore (MoE-routing style).

Strategy (routed / expert-grouped, data-parallel over atoms):
  * Concatenate frag1/frag2/complex atoms into one list with a per-atom
    sign (+1 complex, -1 frags); the answer is
        out[b] = sum_n sign[n] * MLP_{z[n]}(x[n])   (+ bout correction)
  * Sort atoms by type on the host, pad each type group to a multiple of
    8*128 and give each core 1/8 of every type group -> every core runs
    the IDENTICAL instruction schedule (SPMD) on different data.
  * L1/L2 run feature-major ([feature, atom]) in bf16 (1 cyc/row,
    fp32 PSUM):  h^T = relu(W^T_chunk @ x^T_chunk + b)
  * L3 is computed TRANSPOSED per 128-atom tile: the PE contracts the
    h2-feature axis with h2 as the stationary operand,
        p3T[atom, k] = sum_j h2[j, atom] * W2[j, k]
    so after the relu h3T sits atom-major in SBUF with zero extra
    transpose cost (moving cycles are identical to the normal L3).
  * Reduction: per tile one matmul G'[b, k] += S_j^T @ h3T_j (ap=128)
    accumulates the signed per-batch sums in PSUM across each type
    (S = signed batch-mask, host-built).  At a type boundary a single
    DVE tensor_tensor_reduce folds Wout in:  res[b, t] = sum_k
    G'[b, k] * wo_t[k].  No per-atom energy row ever exists.
  * Host sums the 8 per-core partials over types/cores and adds the
    tiny bout correction.
"""

import os
import sys

sys.path.insert(0, "/opt/trn_rl_repo")

import numpy as np
import ml_dtypes

import concourse.bass as bass
import concourse.tile as tile
from concourse import bacc, mybir
from concourse.bass_utils import run_bass_kernel_spmd

# Problem constants (hardcoded per the self-contained-kernel contract).
B = 16
F = 256
H1, H2, H3 = 256, 256, 128
T = 5
NCORES = 8
PTILE = 128  # atoms per tile (partition dim)
SUPER = 4    # tiles per supertile -> moving dim N = 512
CHUNK_TILES = 16  # tiles per x chunk (2048 atoms)
N_WARMUP = int(os.environ.get("KWARM", "3"))
KDBG = set(filter(None, os.environ.get("KDBG", "").split(",")))

MM_MODE = "bf16-l3t"   # informational (printed by test.py)
TRACE = False          # test.py sets this for profiling runs
LAST_RESULTS = None    # test.py reads exec_time_ns from here

_F32 = mybir.dt.float32
_BF16 = mybir.dt.bfloat16

# CONSTW column layout per type t:
#   +0    w0 chunks (k,m) at (k*2+m)*128
#   +512  w1 chunks (k,m)
#   +1024 w2 chunks (k)
_TBLK = 1280
_WCOLS = T * _TBLK
# CONSTF: 25 bias cols, then per-type wo columns (f32 [128, 1]) at 26+t
_FCOLS = 26 + T


def _schedule(k_t):
    """supertile schedule [(t, ntiles)] and chunk grouping [n_supertiles]."""
    sched = []
    for t in range(T):
        left = int(k_t[t])
        while left > 0:
            nt = min(SUPER, left)
            sched.append((t, nt))
            left -= nt
    chunks = []
    cur, cur_tiles = 0, 0
    cap = SUPER  # small first chunk so compute starts early
    for _, nt in sched:
        if cur_tiles + nt > cap and cur > 0:
            chunks.append(cur)
            cur, cur_tiles = 0, 0
            cap = CHUNK_TILES
        cur += 1
        cur_tiles += nt
    if cur:
        chunks.append(cur)
    return sched, chunks


def _build(k_t, n_core, has_b2):
    """Build the (SPMD-uniform) Bass program for one core."""
    ntt = n_core // PTILE
    relu = mybir.ActivationFunctionType.Relu
    sched, chunks = _schedule(k_t)

    nc = bacc.Bacc()
    xT_d = nc.dram_tensor("xT", [F, n_core], _BF16, kind="ExternalInput")
    CW_d = nc.dram_tensor("CONSTW", [PTILE, _WCOLS], _BF16, kind="ExternalInput")
    CF_d = nc.dram_tensor("CONSTF", [PTILE, _FCOLS], _F32, kind="ExternalInput")
    S2_d = nc.dram_tensor("S2", [PTILE, ntt * B], _BF16, kind="ExternalInput")
    B2_d = None
    if has_b2:
        B2_d = nc.dram_tensor("B2R", [1, PTILE + T * H3], _BF16,
                              kind="ExternalInput")
    out_d = nc.dram_tensor("res", [1, B], _F32, kind="ExternalOutput")

    active = sorted({t for t, _ in sched})
    last_st = {t: max(i for i, (tt, _) in enumerate(sched) if tt == t)
               for t in active}

    with tile.TileContext(nc) as tc:
        with (
            tc.tile_pool(name="const", bufs=1) as cpool,
            tc.tile_pool(name="x", bufs=4) as xpool,
            tc.tile_pool(name="h", bufs=4) as hpool,
            tc.tile_pool(name="h3t", bufs=6) as htpool,
            tc.tile_pool(name="gs", bufs=3) as gspool,
            tc.tile_pool(name="pl1", bufs=2, space="PSUM") as pl1,
            tc.tile_pool(name="pl2", bufs=2, space="PSUM") as pl2,
            tc.tile_pool(name="pl3", bufs=2, space="PSUM") as pl3,
            tc.tile_pool(name="pg", bufs=1, space="PSUM") as pgpool,
            tc.tile_pool(name="pres", bufs=1, space="PSUM") as prespool,
        ):
            # ---- activation-table preload + PE warmup while the
            # constant/x DMAs stream in. fp32 warmup (4 cyc/row); values
            # never read.
            warm1 = cpool.tile([128, 128], _F32, tag="warm1")
            nc.gpsimd.memset(warm1[:], 0.0)
            nc.scalar.activation(warm1[:, 0:1], warm1[:, 1:2], relu)
            wps = pl1.tile([128, 512], _F32, tag="ph1")
            # short warmups (fp32 runs as 2 passes on hw) start the PE clock
            # ramp as early as the cross-engine semaphore allows; real work
            # begins as soon as the first x chunk + type-0 weights land
            for _ in range(N_WARMUP):
                nc.tensor.matmul(wps[:, 0:128], warm1[:], warm1[:],
                                 start=True, stop=True)

            # ---- first x chunk + constants (sync-queue order matters:
            # compute waits on x0 + type-0 weights first).
            c0tiles = sum(nt for _, nt in sched[:chunks[0]])
            x0c0 = xpool.tile([128, c0tiles * PTILE], _BF16, tag="x0")
            nc.sync.dma_start(x0c0[:], xT_d[0:128, 0:c0tiles * PTILE])
            x1c0 = xpool.tile([128, c0tiles * PTILE], _BF16, tag="x1")
            nc.sync.dma_start(x1c0[:], xT_d[128:256, 0:c0tiles * PTILE])

            # one SBUF tile per type: tile-granular dependency tracking
            # otherwise makes type-0's first matmul wait for ALL weight DMAs
            CWt = {t: cpool.tile([PTILE, _TBLK], _BF16, tag=f"CW{t}",
                                 name=f"CW{t}")
                   for t in active}
            t0 = active[0]
            nc.sync.dma_start(CWt[t0][:], CW_d[:, t0 * _TBLK:(t0 + 1) * _TBLK])
            CF = cpool.tile([PTILE, _FCOLS], _F32, tag="CF")
            nc.sync.dma_start(CF[:], CF_d[:])
            S2 = cpool.tile([PTILE, ntt * B], _BF16, tag="S2")
            nc.sync.dma_start(S2[:], S2_d[:])
            B2R = None
            if has_b2:
                B2R = cpool.tile([1, PTILE + T * H3], _BF16, tag="B2R")
                nc.sync.dma_start(B2R[:], B2_d[:])

            xc1 = None
            if len(chunks) > 1:
                c1tiles = sum(nt for _, nt in
                              sched[chunks[0]:chunks[0] + chunks[1]])
                c1off = c0tiles * PTILE
                x0c1 = xpool.tile([128, c1tiles * PTILE], _BF16, tag="x0")
                nc.sync.dma_start(x0c1[:],
                                  xT_d[0:128, c1off:c1off + c1tiles * PTILE])
                x1c1 = xpool.tile([128, c1tiles * PTILE], _BF16, tag="x1")
                nc.sync.dma_start(x1c1[:],
                                  xT_d[128:256, c1off:c1off + c1tiles * PTILE])
                xc1 = (x0c1, x1c1)

            for t in active[1:]:
                nc.sync.dma_start(CWt[t][:],
                                  CW_d[:, t * _TBLK:(t + 1) * _TBLK])

            def w0(t, k, m):
                c = (k * 2 + m) * 128
                return CWt[t][:, c:c + 128]

            def w1(t, k, m):
                c = 512 + (k * 2 + m) * 128
                return CWt[t][:, c:c + 128]

            def w2(t, k):
                c = 1024 + k * 128
                return CWt[t][:, c:c + 128]

            def bias(t, c):
                return CF[:, t * 5 + c:t * 5 + c + 1]

            def wo(t):
                return CF[:, 26 + t:27 + t]

            pres = prespool.tile([1, B], _F32, tag="pres")

            # ---- main loop over chunks of supertiles ----
            si = 0       # supertile index
            col = 0      # atom column offset
            jg = 0       # global tile index
            G = None     # current type's PSUM accumulator [B, H3]
            g_open = False
            for ci, n_super in enumerate(chunks):
                csts = sched[si:si + n_super]
                ctiles = sum(nt for _, nt in csts)
                cN = ctiles * PTILE
                if ci == 0:
                    x0, x1 = x0c0, x1c0
                elif ci == 1 and xc1 is not None:
                    x0, x1 = xc1
                else:
                    x0 = xpool.tile([128, cN], _BF16, tag="x0")
                    nc.sync.dma_start(x0[:], xT_d[0:128, col:col + cN])
                    x1 = xpool.tile([128, cN], _BF16, tag="x1")
                    nc.sync.dma_start(x1[:], xT_d[128:256, col:col + cN])

                ccol = 0
                for k, (t, nt) in enumerate(csts):
                    sti = si + k
                    N = PTILE * nt
                    xs0 = x0[:, ccol:ccol + N]
                    xs1 = x1[:, ccol:ccol + N]

                    def layer(xa, xb, wf, bc, pool, tag):
                        outs = []
                        for m in range(2):
                            p = pool.tile([128, N], _F32, tag=tag)
                            nc.tensor.matmul(p[:], wf(t, 0, m), xa[:],
                                             start=True, stop=False)
                            nc.tensor.matmul(p[:], wf(t, 1, m), xb[:],
                                             start=False, stop=True)
                            h = hpool.tile([128, N], _BF16, tag=f"{tag}_{m}")
                            if m == 0:
                                nc.scalar.activation(h[:], p[:], relu,
                                                     bias=bias(t, bc + m))
                            else:
                                nc.vector.tensor_scalar(
                                    h[:], p[:], bias(t, bc + m), 0.0,
                                    mybir.AluOpType.add, mybir.AluOpType.max)
                            outs.append(h)
                        return outs

                    h1 = layer(xs0, xs1, w0, 0, pl1, "ph1")
                    h2 = layer(h1[0], h1[1], w1, 2, pl2, "ph2")

                    if G is None:
                        G = pgpool.tile([H3, B], _F32, tag="G")
                        g_open = False

                    # L3 transposed per tile + G' accumulation
                    for j in range(nt):
                        jj = jg + j
                        jc = j * PTILE
                        p3 = pl3.tile([128, H3], _F32, tag="p3t")
                        nc.tensor.matmul(p3[:], h2[0][:, jc:jc + PTILE],
                                         w2(t, 0), start=True, stop=False)
                        nc.tensor.matmul(p3[:], h2[1][:, jc:jc + PTILE],
                                         w2(t, 1), start=False,
                                         stop=not has_b2)
                        if has_b2:
                            c0 = PTILE + t * H3
                            nc.tensor.matmul(
                                p3[:], B2R[0:1, 0:PTILE],
                                B2R[0:1, c0:c0 + H3],
                                start=False, stop=True)
                        h3t = htpool.tile([128, H3], _BF16, tag="h3t")
                        if j % 2 == 0:
                            nc.vector.tensor_scalar_max(h3t[:], p3[:], 0.0)
                        else:
                            nc.scalar.activation(h3t[:], p3[:], relu)
                        if "nogp" not in KDBG:
                            nc.tensor.matmul(
                                G[:], h3t[:], S2[:, jj * B:(jj + 1) * B],
                                start=not g_open,
                                stop=(sti == last_st[t] and j == nt - 1),
                                skip_group_check=True)
                            g_open = True

                    if sti == last_st[t]:
                        if "nogp" not in KDBG and "nored" not in KDBG:
                            Gsb = gspool.tile([H3, B], _F32, tag="Gsb")
                            nc.vector.tensor_scalar_add(Gsb[:], G[:], 0.0)
                            nc.tensor.matmul(
                                pres[:], wo(t), Gsb[:],
                                start=(t == active[0]),
                                stop=(t == active[-1]),
                                skip_group_check=True)
                        G = None

                    jg += nt
                    ccol += N
                si += n_super
                col += cN

            # ---- final ----
            res_sb = cpool.tile([1, B], _F32, tag="res")
            if KDBG & {"nogp", "nored"}:
                nc.vector.memset(res_sb[:], 0.0)
            else:
                nc.scalar.copy(res_sb[:], pres[:])
            nc.sync.dma_start(out_d[:], res_sb[:])
    nc.finalize()
    return nc


def kernel(**inputs):
    global LAST_RESULTS
    f1 = np.ascontiguousarray(np.asarray(inputs["frag1_layer"], np.float32))
    f2 = np.ascontiguousarray(np.asarray(inputs["frag2_layer"], np.float32))
    cx = np.ascontiguousarray(np.asarray(inputs["complex_layer"], np.float32))
    z1 = np.asarray(inputs["frag1_z"])
    z2 = np.asarray(inputs["frag2_z"])
    zc = np.asarray(inputs["complex_z"])
    W0 = np.asarray(inputs["W0"], np.float32)
    b0 = np.asarray(inputs["b0"], np.float32)
    W1 = np.asarray(inputs["W1"], np.float32)
    b1 = np.asarray(inputs["b1"], np.float32)
    W2 = np.asarray(inputs["W2"], np.float32)
    b2 = np.asarray(inputs["b2"], np.float32)
    Wout = np.asarray(inputs["Wout"], np.float32)
    bout = np.asarray(inputs["bout"], np.float32)

    np_dt = ml_dtypes.bfloat16
    has_b2 = bool(np.any(b2 != 0.0))

    x_all = np.concatenate([f1, f2, cx], axis=1)          # [B, Na, F]
    z_all = np.concatenate([z1, z2, zc], axis=1)          # [B, Na]
    Bn, Na, _ = x_all.shape
    assert Bn == B
    sgn = np.concatenate([
        np.full(f1.shape[1], -1.0, np.float32),
        np.full(f2.shape[1], -1.0, np.float32),
        np.full(cx.shape[1], 1.0, np.float32),
    ])

    xf = x_all.reshape(-1, F)
    zf = z_all.reshape(-1).astype(np.int64)
    bidx = np.repeat(np.arange(B), Na)
    sf = np.tile(sgn, B)

    order = np.argsort(zf, kind="stable")
    counts = np.bincount(zf, minlength=T)[:T]
    GRAN = NCORES * PTILE
    padded = -(-counts // GRAN) * GRAN
    k_t = (padded // GRAN).astype(int)
    n_core = int(padded.sum()) // NCORES
    ntt = n_core // PTILE

    # Per-core atom index lists; -1 marks padding.
    per_core = [[] for _ in range(NCORES)]
    pos = 0
    for t in range(T):
        ct, pt = int(counts[t]), int(padded[t])
        idx = order[pos:pos + ct]
        pos += ct
        if pt == 0:
            continue
        ip = np.full(pt, -1, np.int64)
        ip[:ct] = idx
        ip = ip.reshape(NCORES, pt // NCORES)
        for c in range(NCORES):
            per_core[c].append(ip[c])
    idx_cores = np.stack([np.concatenate(l) for l in per_core])  # [NC, n]

    valid = idx_cores >= 0
    safe = np.where(valid, idx_cores, 0)
    xg = xf[safe]
    xg[~valid] = 0.0
    xT = np.ascontiguousarray(xg.transpose(0, 2, 1)).astype(np_dt)  # [NC,F,n]

    # S[c, n, b] = sign * (batch == b)
    S = np.zeros((NCORES, n_core, B), np.float32)
    rows = sf[safe] * valid
    bcols = bidx[safe]
    S[np.arange(NCORES)[:, None], np.arange(n_core)[None, :], bcols] = rows

    # CONSTW: weights packed per type in the _TBLK layout
    CWh = np.zeros((PTILE, _WCOLS), np.float32)
    for t in range(T):
        base = t * _TBLK
        for k in range(2):
            for m in range(2):
                CWh[:, base + (k * 2 + m) * 128:base + (k * 2 + m + 1) * 128] = \
                    W0[t, 128 * k:128 * (k + 1), 128 * m:128 * (m + 1)]
                CWh[:, base + 512 + (k * 2 + m) * 128:
                    base + 512 + (k * 2 + m + 1) * 128] = \
                    W1[t, 128 * k:128 * (k + 1), 128 * m:128 * (m + 1)]
            CWh[:, base + 1024 + k * 128:base + 1024 + (k + 1) * 128] = \
                W2[t, 128 * k:128 * (k + 1), 0:128]
    CWh = np.ascontiguousarray(CWh).astype(np_dt)

    # CONSTF: 25 bias cols + per-type wo rows replicated over B partitions
    CFh = np.zeros((PTILE, _FCOLS), np.float32)
    for t in range(T):
        CFh[:, t * 5 + 0] = b0[t, :128]
        CFh[:, t * 5 + 1] = b0[t, 128:]
        CFh[:, t * 5 + 2] = b1[t, :128]
        CFh[:, t * 5 + 3] = b1[t, 128:]
        CFh[:, t * 5 + 4] = b2[t, :128]
        CFh[:, 26 + t] = Wout[t, :, 0]

    bias_term = np.bincount(bidx, weights=(sf * bout[zf, 0]).astype(np.float64),
                            minlength=B)[:B]

    nc = _build(k_t, n_core, has_b2)
    in_maps = []
    for c in range(NCORES):
        # S2[p, j*B + b] (tile-major)
        s2 = np.ascontiguousarray(
            S[c].reshape(ntt, PTILE, B).transpose(1, 0, 2)
        ).reshape(PTILE, ntt * B).astype(np_dt)
        m = {"xT": xT[c], "CONSTW": CWh, "CONSTF": CFh, "S2": s2}
        if has_b2:
            b2r = np.zeros((1, PTILE + T * H3), np.float32)
            b2r[0, :PTILE] = 1.0
            b2r[0, PTILE:] = b2[:, :H3].reshape(-1)
            m["B2R"] = b2r.astype(np_dt)
        in_maps.append(m)

    kw = {}
    if TRACE:
        kw = dict(trace=True, trace_cores=list(range(NCORES)))
    res = run_bass_kernel_spmd(nc, in_maps, core_ids=list(range(NCORES)), **kw)
    LAST_RESULTS = res

    parts = np.stack([res.results[c]["res"].reshape(B).astype(np.float64)
                      for c in range(NCORES)])
    out = parts.sum(axis=0) + bias_term
    return out.astype(np.float32)[:, None]
